# revision 2
# baseline (speedup 1.0000x reference)
"""Trainium2 Bass kernel for GQA attention (B=4, S=2048, H=576, 9 heads / 3 KV groups, RoPE).

Sharding: 8 cores = (batch b, seq-half) pairs. Each core computes the full
attention output for 1024 query rows of one batch element (keys/values over
the full 2048 positions of that batch element are recomputed locally; no
collectives needed).

Layout strategy: everything stays "transposed" (features on partitions, seq on
free dim):
  QT = wq @ hsT, KT = wk @ hsT (RoPE applied in T space on DVE)
  V natural [s, hv] via lhsT = hsT chunks
  ST[k, q] = KT.T-stationary @ QT  -> exp on ACT -> attnT fp16 in SBUF
  outT[hd, q] = [V | ones].T @ attnT  (ones column gives softmax denominator)
  final^T = woT.T-stationary @ (outT / denom)
Matmul inputs fp16 (fp32 PSUM accumulation), output fp32.
"""

import sys

if "/opt/trn_rl_repo" not in sys.path:
    sys.path.insert(0, "/opt/trn_rl_repo")

import numpy as np

import concourse.bass as bass
import concourse.mybir as mybir
import concourse.tile as tile
from concourse import bacc
from concourse.bass_utils import run_bass_kernel_spmd

F16 = mybir.dt.float16
F32 = mybir.dt.float32
I16 = mybir.dt.int16

# fp16 Schraudolph exp on DVE: bits = A16*s_raw + B16, bitcast int16->fp16.
# 10-bit mantissa => ~0.03% accuracy, interchangeable with ACT's exact exp.
_LOG2E = float(np.log2(np.e))
A16 = 1024.0 * _LOG2E / 8.0  # folds the 1/sqrt(64) score scale
B16 = 15.0 * 1024.0 - 0.5

# whole (pair, qb) softmax slots whose exps run on DVE (engine-pure per
# softmax: any systematic exp bias cancels in the normalization; contiguous
# engine runs let ACT and DVE pipeline across slots)
# DVE tensor_scalar reads large fp32 PSUM values at reduced internal
# precision (~bf16): x184 in log-domain -> 1-2% exp error, value-dependent,
# not cancellable in softmax.  Exp stays on ACT.
DVE_SLOTS = set()

B = 4
S = 2048
SQ = 1024  # query rows per core
H = 576
HP = 640  # hidden padded to 5*128
NH = 9
HD = 64
KV = 192
G = 3
ROPE_THETA = 10000.0
SCALE = 1.0 / 8.0  # 1/sqrt(HD)

NDC = HP // 128  # 5 contraction chunks
NEC = 5  # output feature chunks of QT (4*128 + 64)
NKC = S // 128  # 16 key chunks
# head pairs for processing: (0,1),(2,3),(4,5),(6,7),(8,)
PAIRS = [(0, 1), (2, 3), (4, 5), (6, 7), (8,)]
import os as _os
DEBUG = bool(int(_os.environ.get("KDBG", "0")))


def _rope_tables():
    inv_freq = 1.0 / (ROPE_THETA ** (np.arange(0, HD, 2, dtype=np.float32) / HD))
    t = np.arange(S, dtype=np.float32)
    freqs = np.einsum("i,j->ij", inv_freq, t)  # [32, S]
    cos32 = np.cos(freqs)
    sin32 = np.sin(freqs)
    cos4 = np.tile(cos32, (4, 1))  # [128, S]
    # sin indexed by the *source* rows of the cross-mul (walrus requires both
    # SBUF inputs of a DVE op to share base partition): the lo output reads
    # hi rows (32-63) and needs -sin there; the hi output reads lo rows (0-31)
    # and needs +sin there.
    sinq = np.concatenate([sin32, -sin32, sin32, -sin32], axis=0)  # [128, S]
    return cos4.astype(np.float16), sinq.astype(np.float16)


def _build_bass():
    nc = bacc.Bacc("TRN2", target_bir_lowering=False)

    hsT = nc.declare_dram_parameter("hsT", [HP, S], F16, isOutput=False)
    wqT = nc.declare_dram_parameter("wqT", [HP, H], F16, isOutput=False)
    wkT = nc.declare_dram_parameter("wkT", [HP, KV], F16, isOutput=False)
    wvT = nc.declare_dram_parameter("wvT", [HP, KV], F16, isOutput=False)
    woT = nc.declare_dram_parameter("woT", [H, H], F16, isOutput=False)
    cos4 = nc.declare_dram_parameter("cos4", [128, S], F16, isOutput=False)
    sinq = nc.declare_dram_parameter("sinq", [128, S], F16, isOutput=False)
    out = nc.declare_dram_parameter("o", [H, SQ], F32, isOutput=True)
    dbg = None
    if DEBUG:
        dbg = {
            "dq": nc.declare_dram_parameter("dq", [NEC * 128, SQ], F16, isOutput=True),
            "dk": nc.declare_dram_parameter("dk", [G * 128, S], F16, isOutput=True),
            "dv": nc.declare_dram_parameter("dv", [NKC * 128, 3 * 65], F16, isOutput=True),
            "dat": nc.declare_dram_parameter("dat", [128, 1024], F16, isOutput=True),
            "dav": nc.declare_dram_parameter("dav", [65, 512], F32, isOutput=True),
            "dot": nc.declare_dram_parameter("dot", [NEC * 128, SQ], F16, isOutput=True),
        }

    with tile.TileContext(nc) as tc:
        kernel_body(nc, tc, hsT, wqT, wkT, wvT, woT, cos4, sinq, out, dbg)

    nc.compile()
    return nc


def kernel_body(nc, tc, hsT, wqT, wkT, wvT, woT, cos4, sinq, out, dbg=None):
    import contextlib

    ctx = contextlib.ExitStack()
    with ctx:
        # ---------------- persistent SBUF pools ----------------
        wpool = ctx.enter_context(tc.tile_pool(name="w", bufs=1))
        qtp = ctx.enter_context(tc.tile_pool(name="qt", bufs=1))
        ktp = ctx.enter_context(tc.tile_pool(name="kt", bufs=1))
        vap = ctx.enter_context(tc.tile_pool(name="va", bufs=1))
        otp = ctx.enter_context(tc.tile_pool(name="ot", bufs=1))
        ropep = ctx.enter_context(tc.tile_pool(name="rope", bufs=2))
        attnp = ctx.enter_context(tc.tile_pool(name="attn", bufs=4))
        miscp = ctx.enter_context(tc.tile_pool(name="misc", bufs=3))

        # ---------------- load inputs to SBUF ----------------
        hs_sb = []
        wq_sb = []
        wk_sb = []
        wv_sb = []
        wo_sb = []
        # load order: K-proj inputs first (wk + hs), then wq, wv, wo, tables —
        # so the first projection can start as early as possible
        for dc in range(NDC):
            t = wpool.tile([128, KV], F16, tag=f"wk{dc}", name=f"wk{dc}")
            nc.sync.dma_start(out=t, in_=wkT[dc * 128 : (dc + 1) * 128, :])
            wk_sb.append(t)
            t = wpool.tile([128, S], F16, tag=f"hs{dc}", name=f"hs{dc}")
            nc.sync.dma_start(out=t, in_=hsT[dc * 128 : (dc + 1) * 128, :])
            hs_sb.append(t)
        for dc in range(NDC):
            t = wpool.tile([128, H], F16, tag=f"wq{dc}", name=f"wq{dc}")
            nc.sync.dma_start(out=t, in_=wqT[dc * 128 : (dc + 1) * 128, :])
            wq_sb.append(t)
        for dc in range(NDC):
            t = wpool.tile([128, KV], F16, tag=f"wv{dc}", name=f"wv{dc}")
            nc.sync.dma_start(out=t, in_=wvT[dc * 128 : (dc + 1) * 128, :])
            wv_sb.append(t)
        for ec in range(NEC):
            m = min(128, H - ec * 128)
            t = wpool.tile([128, H], F16, tag=f"wo{ec}", name=f"wo{ec}")
            nc.sync.dma_start(out=t[:m, :], in_=woT[ec * 128 : ec * 128 + m, :])
            wo_sb.append(t)
        cos_sb = wpool.tile([128, S], F16, tag="cos")
        nc.sync.dma_start(out=cos_sb, in_=cos4[:, :])
        sin_sb = wpool.tile([128, S], F16, tag="sin")
        nc.sync.dma_start(out=sin_sb, in_=sinq[:, :])

        # persistent activation tensors
        qt_sb = [qtp.tile([128, SQ], F16, tag=f"qt{c}", name=f"qt{c}") for c in range(NEC)]
        ktd_sb = [ktp.tile([128, S], F16, tag=f"ktd{g}", name=f"ktd{g}") for g in range(G)]
        va_sb = [vap.tile([128, 3 * 65], F16, tag=f"va{kc}", name=f"va{kc}") for kc in range(NKC)]
        ot_sb = [otp.tile([128, SQ], F16, tag=f"ot{c}", name=f"ot{c}") for c in range(NEC)]

        def rope(dst_writes, src, n_heads, cos_ap, sin_ap, width):
            """Apply RoPE to src [n_heads*64, width] fp16 sbuf tile.

            dst_writes: list of (dst_ap, src_row) per 64-row head giving where
            the rotated head goes. cos_ap/sin_ap are [128, width] slices.
            """
            tc_t = ropep.tile([128, width], F16, tag="ropec")
            tt = ropep.tile([128, width], F16, tag="ropet")
            n = n_heads * 64
            nc.vector.tensor_mul(tc_t[:n], src[:n], cos_ap[:n])
            for h2 in range(n_heads):
                b0 = h2 * 64
                nc.vector.tensor_mul(
                    tt[b0 : b0 + 32], src[b0 + 32 : b0 + 64], sin_ap[b0 + 32 : b0 + 64]
                )
                nc.vector.tensor_mul(
                    tt[b0 + 32 : b0 + 64], src[b0 : b0 + 32], sin_ap[b0 : b0 + 32]
                )
            for dst, row in dst_writes:
                nc.vector.tensor_add(dst, tc_t[row : row + 64], tt[row : row + 64])

        # ---------------- projections ----------------
        # Queries are always hsT columns [0, SQ): cores covering the second
        # seq half pass hsT (and cos/sin) rolled by -SQ columns, which leaves
        # attention invariant (sum over all keys) while keeping one module.
        # One PSUM pool for the whole kernel (no phase barriers): tag "big"
        # [128,1024]f32 x3 = 6 banks, avA/avB 1 bank each = 8 banks.
        QO = 0
        with tc.tile_pool(name="ps", bufs=3, space="PSUM") as pj:
            at = pj
            # K projection + rope, one seq piece at a time; piece 0 first so
            # attention kc 0-7 can start as soon as Q chunk 0 is also done.
            def k_proj(piece):
                so = piece * SQ
                for kc_ch, (roff, nh) in enumerate([(0, 2), (128, 1)]):
                    m = nh * 64
                    kps = pj.tile([128, SQ], F32, tag="big", name=f"kps{piece}{kc_ch}")
                    for dc in range(NDC):
                        for sb2 in range(2):
                            nc.tensor.matmul(
                                kps[:m, sb2 * 512 : (sb2 + 1) * 512],
                                lhsT=wk_sb[dc][:, roff : roff + m],
                                rhs=hs_sb[dc][:, so + sb2 * 512 : so + (sb2 + 1) * 512],
                                start=(dc == 0),
                                stop=(dc == NDC - 1),
                            )
                    kraw = ropep.tile([128, SQ], F16, tag="qraw", name="kraw")
                    nc.vector.tensor_copy(kraw[:m], kps[:m])
                    writes = []
                    for h2 in range(nh):
                        g = kc_ch * 2 + h2
                        writes.append((ktd_sb[g][0:64, so : so + SQ], h2 * 64))
                    rope(writes, kraw, nh, cos_sb[:, so : so + SQ], sin_sb[:, so : so + SQ], SQ)
                # duplicate rows 0-63 -> 64-127 (per piece, so kc<8 doesn't wait
                # on the second seq piece) for row-packed score matmuls
                for g in range(G):
                    nc.sync.dma_start(
                        out=ktd_sb[g][64:128, so : so + SQ],
                        in_=ktd_sb[g][0:64, so : so + SQ],
                    )

            def q_proj(c):
                m = min(128, H - c * 128)
                nh = m // 64
                qps = pj.tile([128, SQ], F32, tag="big", name=f"qps{c}")
                for dc in range(NDC):
                    for sb2 in range(2):
                        nc.tensor.matmul(
                            qps[:m, sb2 * 512 : (sb2 + 1) * 512],
                            lhsT=wq_sb[dc][:, c * 128 : c * 128 + m],
                            rhs=hs_sb[dc][:, QO + sb2 * 512 : QO + (sb2 + 1) * 512],
                            start=(dc == 0),
                            stop=(dc == NDC - 1),
                        )
                qraw = ropep.tile([128, SQ], F16, tag="qraw")
                nc.vector.tensor_copy(qraw[:m], qps[:m])
                writes = [
                    (qt_sb[c][h2 * 64 : h2 * 64 + 64, :], h2 * 64) for h2 in range(nh)
                ]
                rope(writes, qraw, nh, cos_sb[:, QO : QO + SQ], sin_sb[:, QO : QO + SQ], SQ)

            def v_proj(kc):
                vps = pj.tile([128, SQ], F32, tag="big", name=f"vps{kc}")
                for dc in range(NDC):
                    nc.tensor.matmul(
                        vps[:, :KV],
                        lhsT=hs_sb[dc][:, kc * 128 : (kc + 1) * 128],
                        rhs=wv_sb[dc][:, :],
                        start=(dc == 0),
                        stop=(dc == NDC - 1),
                    )
                nc.vector.memset(va_sb[kc], 1.0)
                dst = va_sb[kc].rearrange("p (g w) -> p g w", g=G)[:, :, 0:64]
                srcv = vps[:, :KV].rearrange("p (g w) -> p g w", g=G)
                nc.vector.tensor_copy(dst, srcv)

            k_proj(0)
            q_proj(0)
            for kc in range(NKC):
                v_proj(kc)
            k_proj(1)
            for c in range(1, NEC):
                q_proj(c)

            # ---------------- attention ----------------
            for pi, pair in enumerate(PAIRS):
                hA = pair[0]
                gA = hA // 3
                two = len(pair) == 2
                if two:
                    hB = pair[1]
                    gB = hB // 3
                c = hA // 2  # qt chunk index
                for qb in range(2):
                    avA = at.tile([65, 512], F32, tag="avA", bufs=1)
                    avB = at.tile([65, 512], F32, tag="avB", bufs=1, name="avB") if two else None
                    for kc in range(NKC):
                        st = at.tile([128, 1024], F32, tag="big", bufs=3, name="st")
                        nc.tensor.matmul(
                            st[:, 0:512],
                            lhsT=ktd_sb[gA][0:64, kc * 128 : (kc + 1) * 128],
                            rhs=qt_sb[c][0:64, qb * 512 : (qb + 1) * 512],
                            start=True,
                            stop=True,
                        )
                        if two:
                            nc.tensor.matmul(
                                st[:, 512:1024],
                                lhsT=ktd_sb[gB][64:128, kc * 128 : (kc + 1) * 128],
                                rhs=qt_sb[c][64:128, qb * 512 : (qb + 1) * 512],
                                start=True,
                                stop=True,
                            )
                        width = 1024 if two else 512
                        at_t = attnp.tile([128, 1024], F16, tag="at")
                        if (pi, qb) in DVE_SLOTS and two:
                            # DVE Schraudolph exp (offloads the ACT engine)
                            nc.vector.tensor_scalar(
                                out=at_t.bitcast(I16)[:, :width],
                                in0=st[:, :width],
                                scalar1=A16,
                                scalar2=B16,
                                op0=mybir.AluOpType.mult,
                                op1=mybir.AluOpType.add,
                            )
                        else:
                            nc.scalar.activation(
                                at_t[:, :width],
                                st[:, :width],
                                mybir.ActivationFunctionType.Exp,
                                scale=SCALE,
                            )
                        if dbg is not None and pi == 0 and qb == 0 and kc == 0:
                            nc.sync.dma_start(out=dbg["dat"][:, :], in_=at_t[:, :])
                        nc.tensor.matmul(
                            avA,
                            lhsT=va_sb[kc][:, gA * 65 : gA * 65 + 65],
                            rhs=at_t[:, 0:512],
                            start=(kc == 0),
                            stop=(kc == NKC - 1),
                        )
                        if two:
                            nc.tensor.matmul(
                                avB,
                                lhsT=va_sb[kc][:, gB * 65 : gB * 65 + 65],
                                rhs=at_t[:, 512:1024],
                                start=(kc == 0),
                                stop=(kc == NKC - 1),
                            )
                    # normalize: out^T = av[0:64] / av[64]
                    if dbg is not None and pi == 0 and qb == 0:
                        davs = miscp.tile([65, 512], F32, tag="davs", name="davs")
                        nc.scalar.copy(davs, avA)
                        nc.sync.dma_start(out=dbg["dav"][:, :], in_=davs)
                    for idx, (h, av) in enumerate(
                        [(hA, avA)] + ([(hB, avB)] if two else [])
                    ):
                        # custom-DVE ops drop PSUM partition offsets (measured:
                        # reciprocal_approx_fast on av[64:65] read partition 0),
                        # so stage the denominator row through SBUF first.
                        dn = miscp.tile([1, 512], F32, tag="dn")
                        nc.vector.tensor_copy(dn, av[64:65, :])
                        rd = miscp.tile([1, 512], F32, tag="rd")
                        nc.vector.reciprocal_approx_fast(out=rd, in_=dn)
                        bc = miscp.tile([64, 512], F32, tag="bc")
                        nc.gpsimd.partition_broadcast(bc, rd)
                        row = (h % 2) * 64
                        nc.vector.tensor_mul(
                            ot_sb[h // 2][row : row + 64, qb * 512 : (qb + 1) * 512],
                            av[0:64, :],
                            bc,
                        )

            if dbg is not None:
                for c in range(NEC):
                    nc.sync.dma_start(out=dbg["dq"][c * 128 : (c + 1) * 128, :], in_=qt_sb[c])
                    nc.sync.dma_start(out=dbg["dot"][c * 128 : (c + 1) * 128, :], in_=ot_sb[c])
                for g in range(G):
                    nc.sync.dma_start(out=dbg["dk"][g * 128 : (g + 1) * 128, :], in_=ktd_sb[g])
                for kc in range(NKC):
                    nc.sync.dma_start(out=dbg["dv"][kc * 128 : (kc + 1) * 128, :], in_=va_sb[kc])

            # ---------------- output projection ----------------
            for ec in range(NEC):
                m = min(128, H - ec * 128)
                for sb2 in range(2):
                    ft = pj.tile([128, SQ], F32, tag="big", name=f"ft{ec}{sb2}")[:, :512]
                    for cc in range(NEC):
                        k = min(128, H - cc * 128)
                        nc.tensor.matmul(
                            ft[:m, :],
                            lhsT=wo_sb[cc][:k, ec * 128 : ec * 128 + m],
                            rhs=ot_sb[cc][:k, sb2 * 512 : (sb2 + 1) * 512],
                            start=(cc == 0),
                            stop=(cc == NEC - 1),
                        )
                    fts = miscp.tile([128, 512], F32, tag="fts", name="fts")
                    nc.scalar.copy(fts[:m, :], ft[:m, :])
                    nc.sync.dma_start(
                        out=out[ec * 128 : ec * 128 + m, sb2 * 512 : (sb2 + 1) * 512],
                        in_=fts[:m, :],
                    )

_NC_CACHE = {}


def _get_nc():
    if "nc" not in _NC_CACHE:
        _NC_CACHE["nc"] = _build_bass()
    return _NC_CACHE["nc"]


def kernel(hidden_states, wq, wk, wv, wo):
    cos4, sinq = _rope_tables()

    wq16 = np.zeros((HP, H), np.float16)
    wq16[:H] = wq.T.astype(np.float16)
    wk16 = np.zeros((HP, KV), np.float16)
    wk16[:H] = wk.T.astype(np.float16)
    wv16 = np.zeros((HP, KV), np.float16)
    wv16[:H] = wv.T.astype(np.float16)
    wo16 = wo.T.astype(np.float16)

    cos4r = np.roll(cos4, -SQ, axis=1)
    sinqr = np.roll(sinq, -SQ, axis=1)

    in_maps = []
    core_ids = list(range(8))
    for c in core_ids:
        b, half = c // 2, c % 2
        hsT16 = np.zeros((HP, S), np.float16)
        hsT16[:H] = hidden_states[b].T.astype(np.float16)
        if half == 1:
            # roll so this core's queries sit at columns [0, SQ); keys keep
            # their correct rope position via the equally-rolled cos/sin.
            hsT16 = np.roll(hsT16, -SQ, axis=1)
        in_maps.append(
            {
                "hsT": hsT16,
                "wqT": wq16,
                "wkT": wk16,
                "wvT": wv16,
                "woT": wo16,
                "cos4": cos4 if half == 0 else cos4r,
                "sinq": sinq if half == 0 else sinqr,
            }
        )

    global _LAST_IN_MAPS
    _LAST_IN_MAPS = in_maps
    nc = _get_nc()
    res = run_bass_kernel_spmd(nc, in_maps, core_ids=core_ids)

    out = np.empty((B, S, H), np.float32)
    for c in core_ids:
        b, half = c // 2, c % 2
        out[b, half * SQ : (half + 1) * SQ, :] = res.results[c]["o"].T
    return out


if __name__ == "__main__":
    rng = np.random.default_rng(0)
    hs = rng.standard_normal((B, S, H), dtype=np.float32)
    s = 1.0 / np.sqrt(H)
    wq = rng.standard_normal((H, H), dtype=np.float32) * s
    wk = rng.standard_normal((KV, H), dtype=np.float32) * s
    wv = rng.standard_normal((KV, H), dtype=np.float32) * s
    wo = rng.standard_normal((H, H), dtype=np.float32) * s
    o = kernel(hidden_states=hs, wq=wq, wk=wk, wv=wv, wo=wo)
    print(o.shape, o.dtype, np.abs(o).mean())



# revision 4
# speedup vs baseline: 1.2417x; 1.2417x over previous
"""Trainium2 Bass kernel for GQA attention (B=4, S=2048, H=576, 9 heads / 3 KV groups, RoPE).

Sharding: 8 cores = (batch b, seq-half) pairs. Each core computes the full
attention output for 1024 query rows of one batch element (keys/values over
the full 2048 positions of that batch element are recomputed locally; no
collectives needed).

Layout strategy: everything stays "transposed" (features on partitions, seq on
free dim):
  QT = wq @ hsT, KT = wk @ hsT (RoPE applied in T space on DVE)
  V natural [s, hv] via lhsT = hsT chunks
  ST[k, q] = KT.T-stationary @ QT  -> exp on ACT -> attnT fp16 in SBUF
  outT[hd, q] = [V | ones].T @ attnT  (ones column gives softmax denominator)
  final^T = woT.T-stationary @ (outT / denom)
Matmul inputs fp16 (fp32 PSUM accumulation), output fp32.
"""

import sys

if "/opt/trn_rl_repo" not in sys.path:
    sys.path.insert(0, "/opt/trn_rl_repo")

import numpy as np

import concourse.bass as bass
import concourse.mybir as mybir
import concourse.tile as tile
from concourse import bacc
from concourse.bass_utils import run_bass_kernel_spmd

F16 = mybir.dt.float16
F32 = mybir.dt.float32
I16 = mybir.dt.int16

# fp16 Schraudolph exp on DVE: bits = A16*s_raw + B16, bitcast int16->fp16.
# 10-bit mantissa => ~0.03% accuracy, interchangeable with ACT's exact exp.
_LOG2E = float(np.log2(np.e))
A16 = 1024.0 * _LOG2E / 8.0  # folds the 1/sqrt(64) score scale
B16 = 15.0 * 1024.0 - 0.5

# whole (pair, qb) softmax slots whose exps run on DVE (engine-pure per
# softmax: any systematic exp bias cancels in the normalization; contiguous
# engine runs let ACT and DVE pipeline across slots)
# DVE tensor_scalar reads large fp32 PSUM values at reduced internal
# precision (~bf16): x184 in log-domain -> 1-2% exp error, value-dependent,
# not cancellable in softmax.  Exp stays on ACT.
DVE_SLOTS = set()

B = 4
S = 2048
SQ = 1024  # query rows per core
H = 576
HP = 640  # hidden padded to 5*128
NH = 9
HD = 64
KV = 192
G = 3
ROPE_THETA = 10000.0
SCALE = 1.0 / 8.0  # 1/sqrt(HD)

NDC = HP // 128  # 5 contraction chunks
NEC = 5  # output feature chunks of QT (4*128 + 64)
NKC = S // 128  # 16 key chunks
# head pairs for processing: (0,1),(2,3),(4,5),(6,7),(8,)
PAIRS = [(0, 1), (2, 3), (4, 5), (6, 7), (8,)]
import os as _os
DEBUG = bool(int(_os.environ.get("KDBG", "0")))


def _rope_tables():
    inv_freq = 1.0 / (ROPE_THETA ** (np.arange(0, HD, 2, dtype=np.float32) / HD))
    t = np.arange(S, dtype=np.float32)
    freqs = np.einsum("i,j->ij", inv_freq, t)  # [32, S]
    cos32 = np.cos(freqs)
    sin32 = np.sin(freqs)
    cos4 = np.tile(cos32, (4, 1))  # [128, S]
    # sin indexed by the *source* rows of the cross-mul (walrus requires both
    # SBUF inputs of a DVE op to share base partition): the lo output reads
    # hi rows (32-63) and needs -sin there; the hi output reads lo rows (0-31)
    # and needs +sin there.
    sinq = np.concatenate([sin32, -sin32, sin32, -sin32], axis=0)  # [128, S]
    return cos4.astype(np.float16), sinq.astype(np.float16)


def _build_bass():
    nc = bacc.Bacc("TRN2", target_bir_lowering=False)

    hsT = nc.declare_dram_parameter("hsT", [HP, S], F16, isOutput=False)
    wqT = nc.declare_dram_parameter("wqT", [HP, H], F16, isOutput=False)
    wkT = nc.declare_dram_parameter("wkT", [HP, KV], F16, isOutput=False)
    wvT = nc.declare_dram_parameter("wvT", [HP, KV], F16, isOutput=False)
    woT = nc.declare_dram_parameter("woT", [H, H], F16, isOutput=False)
    cos4 = nc.declare_dram_parameter("cos4", [128, S], F16, isOutput=False)
    sinq = nc.declare_dram_parameter("sinq", [128, S], F16, isOutput=False)
    out = nc.declare_dram_parameter("o", [H, SQ], F32, isOutput=True)
    dbg = None
    if DEBUG:
        dbg = {
            "dq": nc.declare_dram_parameter("dq", [NEC * 128, SQ], F16, isOutput=True),
            "dk": nc.declare_dram_parameter("dk", [G * 128, S], F16, isOutput=True),
            "dv": nc.declare_dram_parameter("dv", [NKC * 128, 3 * 65], F16, isOutput=True),
            "dat": nc.declare_dram_parameter("dat", [128, 1024], F16, isOutput=True),
            "dav": nc.declare_dram_parameter("dav", [65, 512], F32, isOutput=True),
            "dot": nc.declare_dram_parameter("dot", [NEC * 128, SQ], F16, isOutput=True),
        }

    with tile.TileContext(nc) as tc:
        kernel_body(nc, tc, hsT, wqT, wkT, wvT, woT, cos4, sinq, out, dbg)

    nc.compile()
    return nc


def kernel_body(nc, tc, hsT, wqT, wkT, wvT, woT, cos4, sinq, out, dbg=None):
    import contextlib

    ctx = contextlib.ExitStack()
    with ctx:
        # ---------------- persistent SBUF pools ----------------
        wpool = ctx.enter_context(tc.tile_pool(name="w", bufs=1))
        qtp = ctx.enter_context(tc.tile_pool(name="qt", bufs=1))
        ktp = ctx.enter_context(tc.tile_pool(name="kt", bufs=1))
        vap = ctx.enter_context(tc.tile_pool(name="va", bufs=1))
        otp = ctx.enter_context(tc.tile_pool(name="ot", bufs=1))
        ropep = ctx.enter_context(tc.tile_pool(name="rope", bufs=2))
        attnp = ctx.enter_context(tc.tile_pool(name="attn", bufs=4))
        miscp = ctx.enter_context(tc.tile_pool(name="misc", bufs=3))

        # ---------------- load inputs to SBUF ----------------
        hs_sb = []
        wq_sb = []
        wk_sb = []
        wv_sb = []
        wo_sb = []
        # load order follows first-use: wk/hs pairs (K projection), rope
        # tables (K rope at ~9us), then wq (Q proj), wv, wo
        for dc in range(NDC):
            t = wpool.tile([128, KV], F16, tag=f"wk{dc}", name=f"wk{dc}")
            nc.sync.dma_start(out=t, in_=wkT[dc * 128 : (dc + 1) * 128, :])
            wk_sb.append(t)
            t = wpool.tile([128, S], F16, tag=f"hs{dc}", name=f"hs{dc}")
            nc.sync.dma_start(out=t, in_=hsT[dc * 128 : (dc + 1) * 128, :])
            hs_sb.append(t)
        cos_sb = wpool.tile([128, S], F16, tag="cos")
        nc.sync.dma_start(out=cos_sb, in_=cos4[:, :])
        sin_sb = wpool.tile([128, S], F16, tag="sin")
        nc.sync.dma_start(out=sin_sb, in_=sinq[:, :])
        for dc in range(NDC):
            t = wpool.tile([128, H], F16, tag=f"wq{dc}", name=f"wq{dc}")
            nc.sync.dma_start(out=t, in_=wqT[dc * 128 : (dc + 1) * 128, :])
            wq_sb.append(t)
        for dc in range(NDC):
            t = wpool.tile([128, KV], F16, tag=f"wv{dc}", name=f"wv{dc}")
            nc.sync.dma_start(out=t, in_=wvT[dc * 128 : (dc + 1) * 128, :])
            wv_sb.append(t)
        for ec in range(NEC):
            m = min(128, H - ec * 128)
            t = wpool.tile([128, H], F16, tag=f"wo{ec}", name=f"wo{ec}")
            nc.sync.dma_start(out=t[:m, :], in_=woT[ec * 128 : ec * 128 + m, :])
            wo_sb.append(t)

        # persistent activation tensors
        qt_sb = [qtp.tile([128, SQ], F16, tag=f"qt{c}", name=f"qt{c}") for c in range(NEC)]
        ktd_sb = [ktp.tile([128, S], F16, tag=f"ktd{g}", name=f"ktd{g}") for g in range(G)]
        va_sb = [vap.tile([128, 3 * 65], F16, tag=f"va{kc}", name=f"va{kc}") for kc in range(NKC)]
        ot_sb = [otp.tile([128, SQ], F16, tag=f"ot{c}", name=f"ot{c}") for c in range(NEC)]

        def rope(dst_writes, src, n_heads, cos_ap, sin_ap, width):
            """Apply RoPE to src [n_heads*64, width] fp16 sbuf tile.

            dst_writes: list of (dst_ap, src_row) per 64-row head giving where
            the rotated head goes. cos_ap/sin_ap are [128, width] slices.
            """
            tc_t = ropep.tile([128, width], F16, tag="ropec")
            tt = ropep.tile([128, width], F16, tag="ropet")
            n = n_heads * 64
            nc.vector.tensor_mul(tc_t[:n], src[:n], cos_ap[:n])
            for h2 in range(n_heads):
                b0 = h2 * 64
                nc.vector.tensor_mul(
                    tt[b0 : b0 + 32], src[b0 + 32 : b0 + 64], sin_ap[b0 + 32 : b0 + 64]
                )
                nc.vector.tensor_mul(
                    tt[b0 + 32 : b0 + 64], src[b0 : b0 + 32], sin_ap[b0 : b0 + 32]
                )
            for dst, row in dst_writes:
                nc.vector.tensor_add(dst, tc_t[row : row + 64], tt[row : row + 64])

        # ---------------- projections ----------------
        # Queries are always hsT columns [0, SQ): cores covering the second
        # seq half pass hsT (and cos/sin) rolled by -SQ columns, which leaves
        # attention invariant (sum over all keys) while keeping one module.
        # One PSUM pool for the whole kernel (no phase barriers): tag "big"
        # [128,1024]f32 x3 = 6 banks, avA/avB 1 bank each = 8 banks.
        QO = 0
        with tc.tile_pool(name="ps", bufs=3, space="PSUM") as pj:
            at = pj
            # K projection + rope, one seq piece at a time; piece 0 first so
            # attention kc 0-7 can start as soon as Q chunk 0 is also done.
            def k_proj(piece):
                so = piece * SQ
                for kc_ch, (roff, nh) in enumerate([(0, 2), (128, 1)]):
                    m = nh * 64
                    kps = pj.tile([128, SQ], F32, tag="big", name=f"kps{piece}{kc_ch}")
                    for dc in range(NDC):
                        for sb2 in range(2):
                            nc.tensor.matmul(
                                kps[:m, sb2 * 512 : (sb2 + 1) * 512],
                                lhsT=wk_sb[dc][:, roff : roff + m],
                                rhs=hs_sb[dc][:, so + sb2 * 512 : so + (sb2 + 1) * 512],
                                start=(dc == 0),
                                stop=(dc == NDC - 1),
                            )
                    kraw = ropep.tile([128, SQ], F16, tag="qraw", name="kraw")
                    nc.vector.tensor_copy(kraw[:m], kps[:m])
                    writes = []
                    for h2 in range(nh):
                        g = kc_ch * 2 + h2
                        writes.append((ktd_sb[g][0:64, so : so + SQ], h2 * 64))
                    rope(writes, kraw, nh, cos_sb[:, so : so + SQ], sin_sb[:, so : so + SQ], SQ)
                # duplicate rows 0-63 -> 64-127 (per piece, so kc<8 doesn't wait
                # on the second seq piece) for row-packed score matmuls
                for g in range(G):
                    nc.sync.dma_start(
                        out=ktd_sb[g][64:128, so : so + SQ],
                        in_=ktd_sb[g][0:64, so : so + SQ],
                    )

            def q_proj(c):
                m = min(128, H - c * 128)
                nh = m // 64
                qps = pj.tile([128, SQ], F32, tag="big", name=f"qps{c}")
                for dc in range(NDC):
                    for sb2 in range(2):
                        nc.tensor.matmul(
                            qps[:m, sb2 * 512 : (sb2 + 1) * 512],
                            lhsT=wq_sb[dc][:, c * 128 : c * 128 + m],
                            rhs=hs_sb[dc][:, QO + sb2 * 512 : QO + (sb2 + 1) * 512],
                            start=(dc == 0),
                            stop=(dc == NDC - 1),
                        )
                qraw = ropep.tile([128, SQ], F16, tag="qraw")
                nc.vector.tensor_copy(qraw[:m], qps[:m])
                writes = [
                    (qt_sb[c][h2 * 64 : h2 * 64 + 64, :], h2 * 64) for h2 in range(nh)
                ]
                rope(writes, qraw, nh, cos_sb[:, QO : QO + SQ], sin_sb[:, QO : QO + SQ], SQ)

            def v_proj(kc):
                vps = pj.tile([128, SQ], F32, tag="big", name=f"vps{kc}")
                for dc in range(NDC):
                    nc.tensor.matmul(
                        vps[:, :KV],
                        lhsT=hs_sb[dc][:, kc * 128 : (kc + 1) * 128],
                        rhs=wv_sb[dc][:, :],
                        start=(dc == 0),
                        stop=(dc == NDC - 1),
                    )
                nc.vector.memset(va_sb[kc], 1.0)
                dst = va_sb[kc].rearrange("p (g w) -> p g w", g=G)[:, :, 0:64]
                srcv = vps[:, :KV].rearrange("p (g w) -> p g w", g=G)
                # ACT is idle during projections (no exps yet); doing the V
                # evacuation there keeps the psum ring from pacing on DVE,
                # which is busy with rope
                nc.scalar.copy(dst, srcv)

            k_proj(0)
            q_proj(0)
            for kc in range(NKC):
                v_proj(kc)
            k_proj(1)
            for c in range(1, NEC):
                q_proj(c)

            # ---------------- attention ----------------
            for pi, pair in enumerate(PAIRS):
                hA = pair[0]
                gA = hA // 3
                two = len(pair) == 2
                if two:
                    hB = pair[1]
                    gB = hB // 3
                c = hA // 2  # qt chunk index
                for qb in range(2):
                    avA = at.tile([65, 512], F32, tag="avA", bufs=1)
                    avB = at.tile([65, 512], F32, tag="avB", bufs=1, name="avB") if two else None
                    for kc in range(NKC):
                        st = at.tile([128, 1024], F32, tag="big", bufs=3, name="st")
                        nc.tensor.matmul(
                            st[:, 0:512],
                            lhsT=ktd_sb[gA][0:64, kc * 128 : (kc + 1) * 128],
                            rhs=qt_sb[c][0:64, qb * 512 : (qb + 1) * 512],
                            start=True,
                            stop=True,
                        )
                        if two:
                            nc.tensor.matmul(
                                st[:, 512:1024],
                                lhsT=ktd_sb[gB][64:128, kc * 128 : (kc + 1) * 128],
                                rhs=qt_sb[c][64:128, qb * 512 : (qb + 1) * 512],
                                start=True,
                                stop=True,
                            )
                        width = 1024 if two else 512
                        at_t = attnp.tile([128, 1024], F16, tag="at")
                        if (pi, qb) in DVE_SLOTS and two:
                            # DVE Schraudolph exp (offloads the ACT engine)
                            nc.vector.tensor_scalar(
                                out=at_t.bitcast(I16)[:, :width],
                                in0=st[:, :width],
                                scalar1=A16,
                                scalar2=B16,
                                op0=mybir.AluOpType.mult,
                                op1=mybir.AluOpType.add,
                            )
                        else:
                            nc.scalar.activation(
                                at_t[:, :width],
                                st[:, :width],
                                mybir.ActivationFunctionType.Exp,
                                scale=SCALE,
                            )
                        if dbg is not None and pi == 0 and qb == 0 and kc == 0:
                            nc.sync.dma_start(out=dbg["dat"][:, :], in_=at_t[:, :])
                        nc.tensor.matmul(
                            avA,
                            lhsT=va_sb[kc][:, gA * 65 : gA * 65 + 65],
                            rhs=at_t[:, 0:512],
                            start=(kc == 0),
                            stop=(kc == NKC - 1),
                        )
                        if two:
                            nc.tensor.matmul(
                                avB,
                                lhsT=va_sb[kc][:, gB * 65 : gB * 65 + 65],
                                rhs=at_t[:, 512:1024],
                                start=(kc == 0),
                                stop=(kc == NKC - 1),
                            )
                    # normalize: out^T = av[0:64] / av[64]
                    if dbg is not None and pi == 0 and qb == 0:
                        davs = miscp.tile([65, 512], F32, tag="davs", name="davs")
                        nc.scalar.copy(davs, avA)
                        nc.sync.dma_start(out=dbg["dav"][:, :], in_=davs)
                    for idx, (h, av) in enumerate(
                        [(hA, avA)] + ([(hB, avB)] if two else [])
                    ):
                        # custom-DVE ops drop PSUM partition offsets (measured:
                        # reciprocal_approx_fast on av[64:65] read partition 0),
                        # so stage the denominator row through SBUF first.
                        dn = miscp.tile([1, 512], F32, tag="dn")
                        nc.vector.tensor_copy(dn, av[64:65, :])
                        rd = miscp.tile([1, 512], F32, tag="rd")
                        nc.vector.reciprocal_approx_fast(out=rd, in_=dn)
                        bc = miscp.tile([64, 512], F32, tag="bc")
                        nc.gpsimd.partition_broadcast(bc, rd)
                        row = (h % 2) * 64
                        nc.vector.tensor_mul(
                            ot_sb[h // 2][row : row + 64, qb * 512 : (qb + 1) * 512],
                            av[0:64, :],
                            bc,
                        )

            if dbg is not None:
                for c in range(NEC):
                    nc.sync.dma_start(out=dbg["dq"][c * 128 : (c + 1) * 128, :], in_=qt_sb[c])
                    nc.sync.dma_start(out=dbg["dot"][c * 128 : (c + 1) * 128, :], in_=ot_sb[c])
                for g in range(G):
                    nc.sync.dma_start(out=dbg["dk"][g * 128 : (g + 1) * 128, :], in_=ktd_sb[g])
                for kc in range(NKC):
                    nc.sync.dma_start(out=dbg["dv"][kc * 128 : (kc + 1) * 128, :], in_=va_sb[kc])

            # ---------------- output projection ----------------
            for ec in range(NEC):
                m = min(128, H - ec * 128)
                for sb2 in range(2):
                    ft = pj.tile([128, SQ], F32, tag="big", name=f"ft{ec}{sb2}")[:, :512]
                    for cc in range(NEC):
                        k = min(128, H - cc * 128)
                        nc.tensor.matmul(
                            ft[:m, :],
                            lhsT=wo_sb[cc][:k, ec * 128 : ec * 128 + m],
                            rhs=ot_sb[cc][:k, sb2 * 512 : (sb2 + 1) * 512],
                            start=(cc == 0),
                            stop=(cc == NEC - 1),
                        )
                    fts = miscp.tile([128, 512], F32, tag="fts", name="fts")
                    nc.scalar.copy(fts[:m, :], ft[:m, :])
                    nc.sync.dma_start(
                        out=out[ec * 128 : ec * 128 + m, sb2 * 512 : (sb2 + 1) * 512],
                        in_=fts[:m, :],
                    )

_NC_CACHE = {}


def _get_nc():
    if "nc" not in _NC_CACHE:
        _NC_CACHE["nc"] = _build_bass()
    return _NC_CACHE["nc"]


def kernel(hidden_states, wq, wk, wv, wo):
    cos4, sinq = _rope_tables()

    wq16 = np.zeros((HP, H), np.float16)
    wq16[:H] = wq.T.astype(np.float16)
    wk16 = np.zeros((HP, KV), np.float16)
    wk16[:H] = wk.T.astype(np.float16)
    wv16 = np.zeros((HP, KV), np.float16)
    wv16[:H] = wv.T.astype(np.float16)
    wo16 = wo.T.astype(np.float16)

    cos4r = np.roll(cos4, -SQ, axis=1)
    sinqr = np.roll(sinq, -SQ, axis=1)

    in_maps = []
    core_ids = list(range(8))
    for c in core_ids:
        b, half = c // 2, c % 2
        hsT16 = np.zeros((HP, S), np.float16)
        hsT16[:H] = hidden_states[b].T.astype(np.float16)
        if half == 1:
            # roll so this core's queries sit at columns [0, SQ); keys keep
            # their correct rope position via the equally-rolled cos/sin.
            hsT16 = np.roll(hsT16, -SQ, axis=1)
        in_maps.append(
            {
                "hsT": hsT16,
                "wqT": wq16,
                "wkT": wk16,
                "wvT": wv16,
                "woT": wo16,
                "cos4": cos4 if half == 0 else cos4r,
                "sinq": sinq if half == 0 else sinqr,
            }
        )

    global _LAST_IN_MAPS
    _LAST_IN_MAPS = in_maps
    nc = _get_nc()
    res = run_bass_kernel_spmd(nc, in_maps, core_ids=core_ids)

    out = np.empty((B, S, H), np.float32)
    for c in core_ids:
        b, half = c // 2, c % 2
        out[b, half * SQ : (half + 1) * SQ, :] = res.results[c]["o"].T
    return out


if __name__ == "__main__":
    rng = np.random.default_rng(0)
    hs = rng.standard_normal((B, S, H), dtype=np.float32)
    s = 1.0 / np.sqrt(H)
    wq = rng.standard_normal((H, H), dtype=np.float32) * s
    wk = rng.standard_normal((KV, H), dtype=np.float32) * s
    wv = rng.standard_normal((KV, H), dtype=np.float32) * s
    wo = rng.standard_normal((H, H), dtype=np.float32) * s
    o = kernel(hidden_states=hs, wq=wq, wk=wk, wv=wv, wo=wo)
    print(o.shape, o.dtype, np.abs(o).mean())



# revision 5
# speedup vs baseline: 1.2702x; 1.0229x over previous
"""Trainium2 Bass kernel for GQA attention (B=4, S=2048, H=576, 9 heads / 3 KV groups, RoPE).

Sharding: 8 cores = (batch b, seq-half) pairs. Each core computes the full
attention output for 1024 query rows of one batch element (keys/values over
the full 2048 positions of that batch element are recomputed locally; no
collectives needed).

Layout strategy: everything stays "transposed" (features on partitions, seq on
free dim):
  QT = wq @ hsT, KT = wk @ hsT (RoPE applied in T space on DVE)
  V natural [s, hv] via lhsT = hsT chunks
  ST[k, q] = KT.T-stationary @ QT  -> exp on ACT -> attnT fp16 in SBUF
  outT[hd, q] = [V | ones].T @ attnT  (ones column gives softmax denominator)
  final^T = woT.T-stationary @ (outT / denom)
Matmul inputs fp16 (fp32 PSUM accumulation), output fp32.
"""

import sys

if "/opt/trn_rl_repo" not in sys.path:
    sys.path.insert(0, "/opt/trn_rl_repo")

import numpy as np

import concourse.bass as bass
import concourse.mybir as mybir
import concourse.tile as tile
from concourse import bacc
from concourse.bass_utils import run_bass_kernel_spmd

F16 = mybir.dt.float16
F32 = mybir.dt.float32
I16 = mybir.dt.int16

# fp16 Schraudolph exp on DVE: bits = A16*s_raw + B16, bitcast int16->fp16.
# 10-bit mantissa => ~0.03% accuracy, interchangeable with ACT's exact exp.
_LOG2E = float(np.log2(np.e))
A16 = 1024.0 * _LOG2E / 8.0  # folds the 1/sqrt(64) score scale
B16 = 15.0 * 1024.0 - 0.5

# whole (pair, qb) softmax slots whose exps run on DVE (engine-pure per
# softmax: any systematic exp bias cancels in the normalization; contiguous
# engine runs let ACT and DVE pipeline across slots)
# DVE tensor_scalar reads large fp32 PSUM values at reduced internal
# precision (~bf16): x184 in log-domain -> 1-2% exp error, value-dependent,
# not cancellable in softmax.  Exp stays on ACT.
DVE_SLOTS = set()

B = 4
S = 2048
SQ = 1024  # query rows per core
H = 576
HP = 640  # hidden padded to 5*128
NH = 9
HD = 64
KV = 192
G = 3
ROPE_THETA = 10000.0
SCALE = 1.0 / 8.0  # 1/sqrt(HD)

NDC = HP // 128  # 5 contraction chunks
NEC = 5  # output feature chunks of QT (4*128 + 64)
NKC = S // 128  # 16 key chunks
# head pairs for processing: (0,1),(2,3),(4,5),(6,7),(8,)
PAIRS = [(0, 1), (2, 3), (4, 5), (6, 7), (8,)]
import os as _os
DEBUG = bool(int(_os.environ.get("KDBG", "0")))


def _rope_tables():
    inv_freq = 1.0 / (ROPE_THETA ** (np.arange(0, HD, 2, dtype=np.float32) / HD))
    t = np.arange(S, dtype=np.float32)
    freqs = np.einsum("i,j->ij", inv_freq, t)  # [32, S]
    cos32 = np.cos(freqs)
    sin32 = np.sin(freqs)
    cos4 = np.tile(cos32, (4, 1))  # [128, S]
    # sin indexed by the *source* rows of the cross-mul (walrus requires both
    # SBUF inputs of a DVE op to share base partition): the lo output reads
    # hi rows (32-63) and needs -sin there; the hi output reads lo rows (0-31)
    # and needs +sin there.
    sinq = np.concatenate([sin32, -sin32, sin32, -sin32], axis=0)  # [128, S]
    return cos4.astype(np.float16), sinq.astype(np.float16)


def _build_bass():
    nc = bacc.Bacc("TRN2", target_bir_lowering=False)

    hsT = nc.declare_dram_parameter("hsT", [HP, S], F16, isOutput=False)
    wqT = nc.declare_dram_parameter("wqT", [HP, H], F16, isOutput=False)
    wkT = nc.declare_dram_parameter("wkT", [HP, KV], F16, isOutput=False)
    wvT = nc.declare_dram_parameter("wvT", [HP, KV], F16, isOutput=False)
    woT = nc.declare_dram_parameter("woT", [H, H], F16, isOutput=False)
    cos4 = nc.declare_dram_parameter("cos4", [128, S], F16, isOutput=False)
    sinq = nc.declare_dram_parameter("sinq", [128, S], F16, isOutput=False)
    out = nc.declare_dram_parameter("o", [H, SQ], F32, isOutput=True)
    dbg = None
    if DEBUG:
        dbg = {
            "dq": nc.declare_dram_parameter("dq", [NEC * 128, SQ], F16, isOutput=True),
            "dk": nc.declare_dram_parameter("dk", [G * 128, S], F16, isOutput=True),
            "dv": nc.declare_dram_parameter("dv", [NKC * 128, 3 * 65], F16, isOutput=True),
            "dat": nc.declare_dram_parameter("dat", [128, 1024], F16, isOutput=True),
            "dav": nc.declare_dram_parameter("dav", [65, 512], F32, isOutput=True),
            "dot": nc.declare_dram_parameter("dot", [NEC * 128, SQ], F16, isOutput=True),
        }

    with tile.TileContext(nc) as tc:
        kernel_body(nc, tc, hsT, wqT, wkT, wvT, woT, cos4, sinq, out, dbg)

    nc.compile()
    return nc


def kernel_body(nc, tc, hsT, wqT, wkT, wvT, woT, cos4, sinq, out, dbg=None):
    import contextlib

    ctx = contextlib.ExitStack()
    with ctx:
        # ---------------- persistent SBUF pools ----------------
        wpool = ctx.enter_context(tc.tile_pool(name="w", bufs=1))
        qtp = ctx.enter_context(tc.tile_pool(name="qt", bufs=1))
        ktp = ctx.enter_context(tc.tile_pool(name="kt", bufs=1))
        vap = ctx.enter_context(tc.tile_pool(name="va", bufs=1))
        otp = ctx.enter_context(tc.tile_pool(name="ot", bufs=1))
        ropep = ctx.enter_context(tc.tile_pool(name="rope", bufs=2))
        attnp = ctx.enter_context(tc.tile_pool(name="attn", bufs=4))
        miscp = ctx.enter_context(tc.tile_pool(name="misc", bufs=3))

        # ---------------- load inputs to SBUF ----------------
        hs_sb = []
        wq_sb = []
        wk_sb = []
        wv_sb = []
        wo_sb = []
        # load order follows first-use: wk/hs pairs (K projection), rope
        # tables (K rope at ~9us), then wq (Q proj), wv, wo
        for dc in range(NDC):
            t = wpool.tile([128, KV], F16, tag=f"wk{dc}", name=f"wk{dc}")
            nc.sync.dma_start(out=t, in_=wkT[dc * 128 : (dc + 1) * 128, :])
            wk_sb.append(t)
            t = wpool.tile([128, S], F16, tag=f"hs{dc}", name=f"hs{dc}")
            nc.sync.dma_start(out=t[:, 0:SQ], in_=hsT[dc * 128 : (dc + 1) * 128, 0:SQ])
            nc.sync.dma_start(out=t[:, SQ:S], in_=hsT[dc * 128 : (dc + 1) * 128, SQ:S])
            hs_sb.append(t)
        cos_sb = wpool.tile([128, S], F16, tag="cos")
        nc.sync.dma_start(out=cos_sb, in_=cos4[:, :])
        sin_sb = wpool.tile([128, S], F16, tag="sin")
        nc.sync.dma_start(out=sin_sb, in_=sinq[:, :])
        for dc in range(NDC):
            t = wpool.tile([128, H], F16, tag=f"wq{dc}", name=f"wq{dc}")
            nc.sync.dma_start(out=t, in_=wqT[dc * 128 : (dc + 1) * 128, :])
            wq_sb.append(t)
        for dc in range(NDC):
            t = wpool.tile([128, KV], F16, tag=f"wv{dc}", name=f"wv{dc}")
            nc.sync.dma_start(out=t, in_=wvT[dc * 128 : (dc + 1) * 128, :])
            wv_sb.append(t)
        for ec in range(NEC):
            m = min(128, H - ec * 128)
            t = wpool.tile([128, H], F16, tag=f"wo{ec}", name=f"wo{ec}")
            nc.sync.dma_start(out=t[:m, :], in_=woT[ec * 128 : ec * 128 + m, :])
            wo_sb.append(t)

        # persistent activation tensors
        qt_sb = [qtp.tile([128, SQ], F16, tag=f"qt{c}", name=f"qt{c}") for c in range(NEC)]
        ktd_sb = [ktp.tile([128, S], F16, tag=f"ktd{g}", name=f"ktd{g}") for g in range(G)]
        va_sb = [vap.tile([128, 3 * 65], F16, tag=f"va{kc}", name=f"va{kc}") for kc in range(NKC)]
        ot_sb = [otp.tile([128, SQ], F16, tag=f"ot{c}", name=f"ot{c}") for c in range(NEC)]

        def rope(dst_writes, src, n_heads, cos_ap, sin_ap, width):
            """Apply RoPE to src [n_heads*64, width] fp16 sbuf tile.

            dst_writes: list of (dst_ap, src_row) per 64-row head giving where
            the rotated head goes. cos_ap/sin_ap are [128, width] slices.
            """
            tc_t = ropep.tile([128, width], F16, tag="ropec")
            tt = ropep.tile([128, width], F16, tag="ropet")
            n = n_heads * 64
            nc.vector.tensor_mul(tc_t[:n], src[:n], cos_ap[:n])
            for h2 in range(n_heads):
                b0 = h2 * 64
                nc.vector.tensor_mul(
                    tt[b0 : b0 + 32], src[b0 + 32 : b0 + 64], sin_ap[b0 + 32 : b0 + 64]
                )
                nc.vector.tensor_mul(
                    tt[b0 + 32 : b0 + 64], src[b0 : b0 + 32], sin_ap[b0 : b0 + 32]
                )
            for dst, row in dst_writes:
                nc.vector.tensor_add(dst, tc_t[row : row + 64], tt[row : row + 64])

        # ---------------- projections ----------------
        # Queries are always hsT columns [0, SQ): cores covering the second
        # seq half pass hsT (and cos/sin) rolled by -SQ columns, which leaves
        # attention invariant (sum over all keys) while keeping one module.
        # One PSUM pool for the whole kernel (no phase barriers): tag "big"
        # [128,1024]f32 x3 = 6 banks, avA/avB 1 bank each = 8 banks.
        QO = 0
        with tc.tile_pool(name="ps", bufs=3, space="PSUM") as pj:
            at = pj
            # K projection + rope, one seq piece at a time; piece 0 first so
            # attention kc 0-7 can start as soon as Q chunk 0 is also done.
            def k_proj(piece):
                so = piece * SQ
                for kc_ch, (roff, nh) in enumerate([(0, 2), (128, 1)]):
                    m = nh * 64
                    kps = pj.tile([128, SQ], F32, tag="big", name=f"kps{piece}{kc_ch}")
                    for dc in range(NDC):
                        for sb2 in range(2):
                            nc.tensor.matmul(
                                kps[:m, sb2 * 512 : (sb2 + 1) * 512],
                                lhsT=wk_sb[dc][:, roff : roff + m],
                                rhs=hs_sb[dc][:, so + sb2 * 512 : so + (sb2 + 1) * 512],
                                start=(dc == 0),
                                stop=(dc == NDC - 1),
                            )
                    kraw = ropep.tile([128, SQ], F16, tag="qraw", name="kraw")
                    nc.scalar.copy(kraw[:m], kps[:m])
                    writes = []
                    for h2 in range(nh):
                        g = kc_ch * 2 + h2
                        writes.append((ktd_sb[g][0:64, so : so + SQ], h2 * 64))
                    rope(writes, kraw, nh, cos_sb[:, so : so + SQ], sin_sb[:, so : so + SQ], SQ)
                # duplicate rows 0-63 -> 64-127 (per piece, so kc<8 doesn't wait
                # on the second seq piece) for row-packed score matmuls
                for g in range(G):
                    nc.sync.dma_start(
                        out=ktd_sb[g][64:128, so : so + SQ],
                        in_=ktd_sb[g][0:64, so : so + SQ],
                    )

            def q_proj(c):
                m = min(128, H - c * 128)
                nh = m // 64
                qps = pj.tile([128, SQ], F32, tag="big", name=f"qps{c}")
                for dc in range(NDC):
                    for sb2 in range(2):
                        nc.tensor.matmul(
                            qps[:m, sb2 * 512 : (sb2 + 1) * 512],
                            lhsT=wq_sb[dc][:, c * 128 : c * 128 + m],
                            rhs=hs_sb[dc][:, QO + sb2 * 512 : QO + (sb2 + 1) * 512],
                            start=(dc == 0),
                            stop=(dc == NDC - 1),
                        )
                qraw = ropep.tile([128, SQ], F16, tag="qraw")
                nc.scalar.copy(qraw[:m], qps[:m])
                writes = [
                    (qt_sb[c][h2 * 64 : h2 * 64 + 64, :], h2 * 64) for h2 in range(nh)
                ]
                rope(writes, qraw, nh, cos_sb[:, QO : QO + SQ], sin_sb[:, QO : QO + SQ], SQ)

            def v_proj(kc):
                vps = pj.tile([128, SQ], F32, tag="big", name=f"vps{kc}")
                for dc in range(NDC):
                    nc.tensor.matmul(
                        vps[:, :KV],
                        lhsT=hs_sb[dc][:, kc * 128 : (kc + 1) * 128],
                        rhs=wv_sb[dc][:, :],
                        start=(dc == 0),
                        stop=(dc == NDC - 1),
                    )
                nc.vector.memset(va_sb[kc], 1.0)
                dst = va_sb[kc].rearrange("p (g w) -> p g w", g=G)[:, :, 0:64]
                srcv = vps[:, :KV].rearrange("p (g w) -> p g w", g=G)
                # ACT is idle during projections (no exps yet); doing the V
                # evacuation there keeps the psum ring from pacing on DVE,
                # which is busy with rope
                nc.scalar.copy(dst, srcv)

            k_proj(0)
            q_proj(0)
            for kc in range(NKC):
                v_proj(kc)
            k_proj(1)
            for c in range(1, NEC):
                q_proj(c)

            # ---------------- attention ----------------
            for pi, pair in enumerate(PAIRS):
                hA = pair[0]
                gA = hA // 3
                two = len(pair) == 2
                if two:
                    hB = pair[1]
                    gB = hB // 3
                c = hA // 2  # qt chunk index
                for qb in range(2):
                    avA = at.tile([65, 512], F32, tag="avA", bufs=1)
                    avB = at.tile([65, 512], F32, tag="avB", bufs=1, name="avB") if two else None
                    for kc in range(NKC):
                        st = at.tile([128, 1024], F32, tag="big", bufs=3, name="st")
                        nc.tensor.matmul(
                            st[:, 0:512],
                            lhsT=ktd_sb[gA][0:64, kc * 128 : (kc + 1) * 128],
                            rhs=qt_sb[c][0:64, qb * 512 : (qb + 1) * 512],
                            start=True,
                            stop=True,
                        )
                        if two:
                            nc.tensor.matmul(
                                st[:, 512:1024],
                                lhsT=ktd_sb[gB][64:128, kc * 128 : (kc + 1) * 128],
                                rhs=qt_sb[c][64:128, qb * 512 : (qb + 1) * 512],
                                start=True,
                                stop=True,
                            )
                        width = 1024 if two else 512
                        at_t = attnp.tile([128, 1024], F16, tag="at")
                        if (pi, qb) in DVE_SLOTS and two:
                            # DVE Schraudolph exp (offloads the ACT engine)
                            nc.vector.tensor_scalar(
                                out=at_t.bitcast(I16)[:, :width],
                                in0=st[:, :width],
                                scalar1=A16,
                                scalar2=B16,
                                op0=mybir.AluOpType.mult,
                                op1=mybir.AluOpType.add,
                            )
                        else:
                            nc.scalar.activation(
                                at_t[:, :width],
                                st[:, :width],
                                mybir.ActivationFunctionType.Exp,
                                scale=SCALE,
                            )
                        if dbg is not None and pi == 0 and qb == 0 and kc == 0:
                            nc.sync.dma_start(out=dbg["dat"][:, :], in_=at_t[:, :])
                        nc.tensor.matmul(
                            avA,
                            lhsT=va_sb[kc][:, gA * 65 : gA * 65 + 65],
                            rhs=at_t[:, 0:512],
                            start=(kc == 0),
                            stop=(kc == NKC - 1),
                        )
                        if two:
                            nc.tensor.matmul(
                                avB,
                                lhsT=va_sb[kc][:, gB * 65 : gB * 65 + 65],
                                rhs=at_t[:, 512:1024],
                                start=(kc == 0),
                                stop=(kc == NKC - 1),
                            )
                    # normalize: out^T = av[0:64] / av[64]
                    if dbg is not None and pi == 0 and qb == 0:
                        davs = miscp.tile([65, 512], F32, tag="davs", name="davs")
                        nc.scalar.copy(davs, avA)
                        nc.sync.dma_start(out=dbg["dav"][:, :], in_=davs)
                    for idx, (h, av) in enumerate(
                        [(hA, avA)] + ([(hB, avB)] if two else [])
                    ):
                        # custom-DVE ops drop PSUM partition offsets (measured:
                        # reciprocal_approx_fast on av[64:65] read partition 0),
                        # so stage the denominator row through SBUF first.
                        dn = miscp.tile([1, 512], F32, tag="dn")
                        nc.vector.tensor_copy(dn, av[64:65, :])
                        rd = miscp.tile([1, 512], F32, tag="rd")
                        nc.vector.reciprocal_approx_fast(out=rd, in_=dn)
                        bc = miscp.tile([64, 512], F32, tag="bc")
                        nc.gpsimd.partition_broadcast(bc, rd)
                        row = (h % 2) * 64
                        nc.vector.tensor_mul(
                            ot_sb[h // 2][row : row + 64, qb * 512 : (qb + 1) * 512],
                            av[0:64, :],
                            bc,
                        )

            if dbg is not None:
                for c in range(NEC):
                    nc.sync.dma_start(out=dbg["dq"][c * 128 : (c + 1) * 128, :], in_=qt_sb[c])
                    nc.sync.dma_start(out=dbg["dot"][c * 128 : (c + 1) * 128, :], in_=ot_sb[c])
                for g in range(G):
                    nc.sync.dma_start(out=dbg["dk"][g * 128 : (g + 1) * 128, :], in_=ktd_sb[g])
                for kc in range(NKC):
                    nc.sync.dma_start(out=dbg["dv"][kc * 128 : (kc + 1) * 128, :], in_=va_sb[kc])

            # ---------------- output projection ----------------
            for ec in range(NEC):
                m = min(128, H - ec * 128)
                for sb2 in range(2):
                    ft = pj.tile([128, SQ], F32, tag="big", name=f"ft{ec}{sb2}")[:, :512]
                    for cc in range(NEC):
                        k = min(128, H - cc * 128)
                        nc.tensor.matmul(
                            ft[:m, :],
                            lhsT=wo_sb[cc][:k, ec * 128 : ec * 128 + m],
                            rhs=ot_sb[cc][:k, sb2 * 512 : (sb2 + 1) * 512],
                            start=(cc == 0),
                            stop=(cc == NEC - 1),
                        )
                    fts = miscp.tile([128, 512], F32, tag="fts", name="fts")
                    nc.scalar.copy(fts[:m, :], ft[:m, :])
                    nc.sync.dma_start(
                        out=out[ec * 128 : ec * 128 + m, sb2 * 512 : (sb2 + 1) * 512],
                        in_=fts[:m, :],
                    )

_NC_CACHE = {}


def _get_nc():
    if "nc" not in _NC_CACHE:
        _NC_CACHE["nc"] = _build_bass()
    return _NC_CACHE["nc"]


def kernel(hidden_states, wq, wk, wv, wo):
    cos4, sinq = _rope_tables()

    wq16 = np.zeros((HP, H), np.float16)
    wq16[:H] = wq.T.astype(np.float16)
    wk16 = np.zeros((HP, KV), np.float16)
    wk16[:H] = wk.T.astype(np.float16)
    wv16 = np.zeros((HP, KV), np.float16)
    wv16[:H] = wv.T.astype(np.float16)
    wo16 = wo.T.astype(np.float16)

    cos4r = np.roll(cos4, -SQ, axis=1)
    sinqr = np.roll(sinq, -SQ, axis=1)

    in_maps = []
    core_ids = list(range(8))
    for c in core_ids:
        b, half = c // 2, c % 2
        hsT16 = np.zeros((HP, S), np.float16)
        hsT16[:H] = hidden_states[b].T.astype(np.float16)
        if half == 1:
            # roll so this core's queries sit at columns [0, SQ); keys keep
            # their correct rope position via the equally-rolled cos/sin.
            hsT16 = np.roll(hsT16, -SQ, axis=1)
        in_maps.append(
            {
                "hsT": hsT16,
                "wqT": wq16,
                "wkT": wk16,
                "wvT": wv16,
                "woT": wo16,
                "cos4": cos4 if half == 0 else cos4r,
                "sinq": sinq if half == 0 else sinqr,
            }
        )

    global _LAST_IN_MAPS
    _LAST_IN_MAPS = in_maps
    nc = _get_nc()
    res = run_bass_kernel_spmd(nc, in_maps, core_ids=core_ids)

    out = np.empty((B, S, H), np.float32)
    for c in core_ids:
        b, half = c // 2, c % 2
        out[b, half * SQ : (half + 1) * SQ, :] = res.results[c]["o"].T
    return out


if __name__ == "__main__":
    rng = np.random.default_rng(0)
    hs = rng.standard_normal((B, S, H), dtype=np.float32)
    s = 1.0 / np.sqrt(H)
    wq = rng.standard_normal((H, H), dtype=np.float32) * s
    wk = rng.standard_normal((KV, H), dtype=np.float32) * s
    wv = rng.standard_normal((KV, H), dtype=np.float32) * s
    wo = rng.standard_normal((H, H), dtype=np.float32) * s
    o = kernel(hidden_states=hs, wq=wq, wk=wk, wv=wv, wo=wo)
    print(o.shape, o.dtype, np.abs(o).mean())

